# revision 7
# baseline (speedup 1.0000x reference)
"""Trainium2 Bass kernel for nn_NeRFGraph (gnn_message_passing).

Strategy (sharding_hint): nodes are sharded across 8 cores aligned to whole
knn batch groups. 20 groups of 2048 nodes -> cores 0-3 take 3 groups,
cores 4-7 take 2 real groups + 1 dummy (SPMD needs uniform shapes; dummy
output is dropped on the host). MLP weights are replicated (data parallel).

Per-core pipeline, per group g (all layouts are [features(partitions), nodes(free)]):
  1. kNN: scores_ij = 2*x_i.x_j - |x_j|^2 via fp32 matmuls (K=64: 63 coords +
     ones row against [2*coords; -sq]). Self always wins top-1 (score_self =
     |x_i|^2 >= score_ij), so top-3 neighbors = entries 1..3 of the DVE
     max8/max_index scan. Exact fp32 selection matches the jax reference
     (0 flips measured on the real data distribution).
  2. MLP (8 layers + skip at 4) in float32r (TF32-like, 1 cyc/row on PE).
  3. EdgeConv x2, factorized: msg_ij = relu(A_i + C_j) with A = F@(W1a-W1b)+b1,
     C = F@W1b per node; gather C_j with gpsimd ap_gather; then W2 matmul,
     mean over K=3 folded into the next layer's weights (host prescale by 1/3).
  4. rgb = sigmoid(S2 @ w_rgb/3 + b_rgb), sigma from the MLP trunk.
"""

import os
import numpy as np

import concourse.bass as bass
import concourse.tile as tile
from concourse import bacc, mybir, library_config
import concourse.bass_utils as bass_utils

F32 = mybir.dt.float32
F32R = mybir.dt.float32r
U16 = mybir.dt.uint16
I16 = mybir.dt.int16

# problem constants (hardcoded per contract)
B = 40960
NG = 20
DXYZ = 63
DDIR = 27
W = 256
H = 128  # W // 2
KNN = 3

N_CORES = 8
GPC = 3                      # groups per core (SPMD-uniform)
G = B // NG                  # 2048 nodes per group
NODES = GPC * G              # 6144 nodes per core
NT = G // 512                # node tiles of 512 per group
MT = G // 128                # row tiles of 128 per group (knn)
ECCH = 256                   # nodes per edge-conv chunk
ECM = ECCH * KNN             # messages per chunk (768)

_STATE: dict = {}


def _build_nc():
    nc = bacc.Bacc(
        "TRN2",
        target_bir_lowering=False,
        debug=False,
        enable_asserts=False,
        num_devices=N_CORES,
    )
    d = {}

    def inp(name, shape):
        d[name] = nc.dram_tensor(name, list(shape), F32, kind="ExternalInput").ap()

    inp("xt", (91, NODES))        # rows 0-62 xyz, 63 ones, 64-90 dir
    inp("rhsa", (64, NODES))      # rows 0-62 2*xyz, 63 -sq
    inp("w0", (DXYZ, W)); inp("b0", (W, 1))
    inp("wmid", (6, W, W))        # [layer, in, out]
    inp("bmid", (6, W, 1))
    inp("wskip", (DXYZ + W, W)); inp("bskip", (W, 1))
    inp("wfin", (W, W)); inp("bfin", (W, 1))
    inp("wsig", (W, 1)); inp("bsig", (1, 1))
    inp("aw1", (W + DDIR, H)); inp("cw1", (W + DDIR, H)); inp("ab1", (H, 1))
    inp("e1w2", (H, H)); inp("e1b2", (H, 1))
    inp("a2w", (H, H)); inp("c2w", (H, H)); inp("ab2", (H, 1))
    inp("e2w2", (H, H)); inp("e2b2", (H, 1))
    inp("wrgb", (H, 3)); inp("brgb", (3, 1))

    rgb_d = nc.dram_tensor("rgb", [3, NODES], F32, kind="ExternalOutput").ap()
    sig_d = nc.dram_tensor("sig", [1, NODES], F32, kind="ExternalOutput").ap()

    with tile.TileContext(nc) as tc:
        _body(tc, d, rgb_d, sig_d)
    nc.compile()
    return nc


def _body(tc, d, rgb_d, sig_d):
    nc = tc.nc
    ctxs = []

    def pool(name, bufs, space="SBUF"):
        p = tc.tile_pool(name=name, bufs=bufs, space=space)
        ctxs.append(p)
        return p.__enter__()

    wstage = pool("wstage", bufs=2)       # f32 staging for weight rounding
    wp = pool("wp", bufs=1)               # persistent rounded weights / biases
    xp = pool("xp", bufs=2)               # per-group inputs
    xr = pool("xr", bufs=1)               # rounded per-group inputs
    ap_ = pool("ap", bufs=1)              # per-group A1/C1/S1/A2/C2 tables
    ec = pool("ec", bufs=2)               # edge-conv chunk tiles
    hp = pool("hp", bufs=3)               # MLP hidden tiles [128,512]
    sp = pool("sp", bufs=4)               # small tiles (vmax/imax/bias views)
    op = pool("op", bufs=2)               # output staging
    idxp = pool("idxp", bufs=2)
    psp = pool("psp", bufs=2, space="PSUM")
    drp = pool("drp", bufs=2, space="DRAM")

    nc.gpsimd.load_library(library_config.ap_gather)

    # ---- load + round weights to f32r (one-time) ----
    # weights are stored as lists of K-chunk tiles (<=128 partitions each)
    def load_chunks(src_ap, rows, cols, tag):
        """src_ap: DRAM AP [R, cols]; returns list of rounded [r,cols] tiles."""
        out = []
        r0 = 0
        for i, r in enumerate(rows):
            st = wstage.tile([r, cols], F32, tag="wstage")
            nc.sync.dma_start(st[:], src_ap[r0:r0 + r, :])
            wt = wp.tile([r, cols], F32R, tag=f"{tag}_{i}")
            nc.vector.tensor_copy(wt[:], st[:])
            out.append(wt)
            r0 += r
        return out

    def load_b(name, src_ap, rows):
        out = []
        r0 = 0
        for i, r in enumerate(rows):
            bt = wp.tile([r, 1], F32, tag=f"{name}_{i}")
            nc.sync.dma_start(bt[:], src_ap[r0:r0 + r, :])
            out.append(bt)
            r0 += r
        return out

    w0 = load_chunks(d["w0"][:], [DXYZ], W, "w0")[0]
    wmid = [load_chunks(d["wmid"][l], [128, 128], W, f"wmid{l}") for l in range(6)]
    wskip = load_chunks(d["wskip"][:], [DXYZ, 128, 128], W, "wskip")
    wfin = load_chunks(d["wfin"][:], [128, 128], W, "wfin")
    wsig = load_chunks(d["wsig"][:], [128, 128], 1, "wsig")
    aw1 = load_chunks(d["aw1"][:], [128, 128, DDIR], H, "aw1")
    cw1 = load_chunks(d["cw1"][:], [128, 128, DDIR], H, "cw1")
    e1w2 = load_chunks(d["e1w2"][:], [H], H, "e1w2")[0]
    a2w = load_chunks(d["a2w"][:], [H], H, "a2w")[0]
    c2w = load_chunks(d["c2w"][:], [H], H, "c2w")[0]
    e2w2 = load_chunks(d["e2w2"][:], [H], H, "e2w2")[0]
    wrgb = load_chunks(d["wrgb"][:], [H], 3, "wrgb")[0]

    b0 = load_b("b0", d["b0"][:], [128, 128])
    bmid = [load_b(f"bmid{l}", d["bmid"][l], [128, 128]) for l in range(6)]
    bskip = load_b("bskip", d["bskip"][:], [128, 128])
    bfin = load_b("bfin", d["bfin"][:], [128, 128])
    bsig = load_b("bsig", d["bsig"][:], [1])[0]
    ab1 = load_b("ab1", d["ab1"][:], [H])[0]
    e1b2 = load_b("e1b2", d["e1b2"][:], [H])[0]
    ab2 = load_b("ab2", d["ab2"][:], [H])[0]
    e2b2 = load_b("e2b2", d["e2b2"][:], [H])[0]
    brgb = load_b("brgb", d["brgb"][:], [3])[0]

    ACT = mybir.ActivationFunctionType

    for g in range(GPC):
        g0 = g * G
        # ---- load per-group inputs ----
        xt = xp.tile([91, G], F32, tag="xt")
        nc.sync.dma_start(xt[:], d["xt"][:, g0:g0 + G])
        rhsa = xp.tile([64, G], F32, tag="rhsa")
        nc.sync.dma_start(rhsa[:], d["rhsa"][:, g0:g0 + G])
        xtr = xr.tile([64, G], F32R, tag="xtr")
        nc.vector.tensor_copy(xtr[:], xt[0:64, :])
        dirf = xr.tile([DDIR, G], F32, tag="dirf")
        nc.sync.dma_start(dirf[:], d["xt"][64:91, g0:g0 + G])
        dirr = xr.tile([DDIR, G], F32R, tag="dirr")
        nc.vector.tensor_copy(dirr[:], dirf[:])

        # ---- kNN ----
        nbr = drp.tile([G, KNN], U16, tag="nbr")
        for mt in range(MT):
            ps = psp.tile([128, 2048], F32, tag="ps")
            for nt in range(4):
                nc.tensor.matmul(
                    ps[:, nt * 512:(nt + 1) * 512],
                    xt[0:64, mt * 128:(mt + 1) * 128],
                    rhsa[:, nt * 512:(nt + 1) * 512],
                    start=True, stop=True,
                )
            vmax = sp.tile([128, 8], F32, tag="vmax")
            nc.vector.max(vmax[:], ps[:])
            imax = sp.tile([128, 8], U16, tag="imax")
            nc.vector.max_index(imax[:], vmax[:], ps[:])
            nc.sync.dma_start(nbr[mt * 128:(mt + 1) * 128, :], imax[:, 1:4])

        # wrapped gather indices: element m lives at [m % 16, m // 16],
        # replicated into each 16-partition block (one per Q7 core)
        idxw = idxp.tile([128, G * KNN // 16], I16, tag="idxw")
        nbr_w = nbr[:].rearrange("n k -> (n k)").rearrange("(f p) -> p f", p=16)
        for r in range(8):
            nc.sync.dma_start(idxw[16 * r:16 * r + 16, :], nbr_w.bitcast(I16))

        # ---- MLP + A1/C1/sigma ----
        a1 = ap_.tile([H, G], F32, tag="a1")
        c1 = ap_.tile([H, G], F32, tag="c1")
        for nt in range(NT):
            n0 = nt * 512
            sl = slice(n0, n0 + 512)

            def evac(ps, src_sl, dst, bias, func):
                nc.scalar.activation(dst, ps[:, src_sl], func, bias=bias)

            # L0: [63]->256
            ps = psp.tile([128, 2048], F32, tag="ps")
            nc.tensor.matmul(ps[:, 0:512], w0[:, 0:128], xtr[0:DXYZ, sl], start=True, stop=True)
            nc.tensor.matmul(ps[:, 512:1024], w0[:, 128:256], xtr[0:DXYZ, sl], start=True, stop=True)
            h = [hp.tile([128, 512], F32R, tag=f"h{mh}", name=f"h{mh}") for mh in range(2)]
            evac(ps, slice(0, 512), h[0][:], b0[0][:], ACT.Relu)
            evac(ps, slice(512, 1024), h[1][:], b0[1][:], ACT.Relu)

            # layers 1..7
            m = 0
            for layer in range(1, 8):
                ps = psp.tile([128, 2048], F32, tag="ps")
                if layer == 4:
                    bk = bskip
                    for mh in range(2):
                        osl = slice(mh * 512, mh * 512 + 512)
                        msl = slice(mh * 128, mh * 128 + 128)
                        nc.tensor.matmul(ps[:, osl], wskip[0][:, msl],
                                         xtr[0:DXYZ, sl], start=True, stop=False)
                        nc.tensor.matmul(ps[:, osl], wskip[1][:, msl],
                                         h[0][:], start=False, stop=False)
                        nc.tensor.matmul(ps[:, osl], wskip[2][:, msl],
                                         h[1][:], start=False, stop=True)
                else:
                    wk, bk = wmid[m], bmid[m]
                    m += 1
                    for mh in range(2):
                        osl = slice(mh * 512, mh * 512 + 512)
                        msl = slice(mh * 128, mh * 128 + 128)
                        nc.tensor.matmul(ps[:, osl], wk[0][:, msl],
                                         h[0][:], start=True, stop=False)
                        nc.tensor.matmul(ps[:, osl], wk[1][:, msl],
                                         h[1][:], start=False, stop=True)
                hn = [hp.tile([128, 512], F32R, tag=f"h{mh}", name=f"hn{mh}") for mh in range(2)]
                evac(ps, slice(0, 512), hn[0][:], bk[0][:], ACT.Relu)
                evac(ps, slice(512, 1024), hn[1][:], bk[1][:], ACT.Relu)
                h = hn

            # final (no relu) + sigma
            ps = psp.tile([128, 2048], F32, tag="ps")
            for mh in range(2):
                osl = slice(mh * 512, mh * 512 + 512)
                msl = slice(mh * 128, mh * 128 + 128)
                nc.tensor.matmul(ps[:, osl], wfin[0][:, msl],
                                 h[0][:], start=True, stop=False)
                nc.tensor.matmul(ps[:, osl], wfin[1][:, msl],
                                 h[1][:], start=False, stop=True)
            nc.tensor.matmul(ps[0:1, 1024:1536], wsig[0][:], h[0][:], start=True, stop=False)
            nc.tensor.matmul(ps[0:1, 1024:1536], wsig[1][:], h[1][:], start=False, stop=True)
            feat = [hp.tile([128, 512], F32R, tag=f"feat{mh}", name=f"feat{mh}") for mh in range(2)]
            evac(ps, slice(0, 512), feat[0][:], bfin[0][:], ACT.Identity)
            evac(ps, slice(512, 1024), feat[1][:], bfin[1][:], ACT.Identity)
            sgt = op.tile([1, 512], F32, tag="sgt")
            nc.scalar.activation(sgt[:], ps[0:1, 1024:1536], ACT.Identity, bias=bsig[:])
            nc.sync.dma_start(sig_d[:, g0 + n0:g0 + n0 + 512], sgt[:])

            # A1 / C1 over feat(256) + dir(27)
            ps = psp.tile([128, 2048], F32, tag="ps")
            for dst_sl, wt in ((slice(0, 512), aw1), (slice(512, 1024), cw1)):
                nc.tensor.matmul(ps[:, dst_sl], wt[0][:], feat[0][:], start=True, stop=False)
                nc.tensor.matmul(ps[:, dst_sl], wt[1][:], feat[1][:], start=False, stop=False)
                nc.tensor.matmul(ps[:, dst_sl], wt[2][:], dirr[:, sl], start=False, stop=True)
            nc.scalar.activation(a1[:, sl], ps[:, 0:512], ACT.Identity, bias=ab1[:])
            nc.scalar.activation(c1[:, sl], ps[:, 512:1024], ACT.Copy)

        # ---- EdgeConv 1 ----
        s1 = ap_.tile([H, G], F32R, tag="s1")
        for c in range(G // ECCH):
            isl = slice(c * (ECM // 16), (c + 1) * (ECM // 16))
            nsl = slice(c * ECCH, (c + 1) * ECCH)
            g1 = ec.tile([128, ECM], F32, tag="g1")
            nc.gpsimd.ap_gather(g1[:], c1[:], idxw[:, isl], channels=128,
                                num_elems=G, d=1, num_idxs=ECM)
            msg = ec.tile([128, ECM], F32, tag="msg")
            g3 = g1[:].rearrange("p (n k) -> p n k", k=KNN)
            m3 = msg[:].rearrange("p (n k) -> p n k", k=KNN)
            for k in range(KNN):
                nc.vector.tensor_add(m3[:, :, k], g3[:, :, k], a1[:, nsl])
            msgr = ec.tile([128, ECM], F32R, tag="msgr")
            nc.scalar.activation(msgr[:], msg[:], ACT.Relu)
            ps = psp.tile([128, 2048], F32, tag="ps")
            nc.tensor.matmul(ps[:, 0:512], e1w2[:], msgr[:, 0:512], start=True, stop=True)
            nc.tensor.matmul(ps[:, 512:512 + ECM - 512], e1w2[:], msgr[:, 512:ECM], start=True, stop=True)
            h2 = ec.tile([128, ECM], F32, tag="h2")
            nc.scalar.activation(h2[:], ps[:, 0:ECM], ACT.Relu, bias=e1b2[:])
            h23 = h2[:].rearrange("p (n k) -> p n k", k=KNN)
            tmp = ec.tile([128, ECCH], F32, tag="trio")
            nc.vector.tensor_add(tmp[:], h23[:, :, 0], h23[:, :, 1])
            nc.vector.tensor_add(s1[:, nsl], tmp[:], h23[:, :, 2])

        # ---- EdgeConv 2 ----
        a2 = ap_.tile([H, G], F32, tag="a2")
        c2 = ap_.tile([H, G], F32, tag="c2")
        for c in range(NT):
            nsl = slice(c * 512, (c + 1) * 512)
            ps = psp.tile([128, 2048], F32, tag="ps")
            nc.tensor.matmul(ps[:, 0:512], a2w[:], s1[:, nsl], start=True, stop=True)
            nc.tensor.matmul(ps[:, 512:1024], c2w[:], s1[:, nsl], start=True, stop=True)
            nc.scalar.activation(a2[:, nsl], ps[:, 0:512], ACT.Identity, bias=ab2[:])
            nc.scalar.activation(c2[:, nsl], ps[:, 512:1024], ACT.Copy)

        for c in range(G // ECCH):
            isl = slice(c * (ECM // 16), (c + 1) * (ECM // 16))
            nsl = slice(c * ECCH, (c + 1) * ECCH)
            g2 = ec.tile([128, ECM], F32, tag="g1")
            nc.gpsimd.ap_gather(g2[:], c2[:], idxw[:, isl], channels=128,
                                num_elems=G, d=1, num_idxs=ECM)
            msg = ec.tile([128, ECM], F32, tag="msg")
            g3 = g2[:].rearrange("p (n k) -> p n k", k=KNN)
            m3 = msg[:].rearrange("p (n k) -> p n k", k=KNN)
            for k in range(KNN):
                nc.vector.tensor_add(m3[:, :, k], g3[:, :, k], a2[:, nsl])
            msgr = ec.tile([128, ECM], F32R, tag="msgr")
            nc.scalar.activation(msgr[:], msg[:], ACT.Relu)
            ps = psp.tile([128, 2048], F32, tag="ps")
            nc.tensor.matmul(ps[:, 0:512], e2w2[:], msgr[:, 0:512], start=True, stop=True)
            nc.tensor.matmul(ps[:, 512:512 + ECM - 512], e2w2[:], msgr[:, 512:ECM], start=True, stop=True)
            h2 = ec.tile([128, ECM], F32, tag="h2")
            nc.scalar.activation(h2[:], ps[:, 0:ECM], ACT.Relu, bias=e2b2[:])
            h23 = h2[:].rearrange("p (n k) -> p n k", k=KNN)
            tmp = ec.tile([128, ECCH], F32, tag="trio")
            nc.vector.tensor_add(tmp[:], h23[:, :, 0], h23[:, :, 1])
            s2 = ec.tile([128, ECCH], F32R, tag="s2")
            nc.vector.tensor_add(s2[:], tmp[:], h23[:, :, 2])
            # rgb (same psum tile, different banks than the W2 region)
            nc.tensor.matmul(ps[0:3, 1024:1024 + ECCH], wrgb[:], s2[:], start=True, stop=True)
            rgt = op.tile([3, ECCH], F32, tag="rgt")
            nc.scalar.activation(rgt[:], ps[0:3, 1024:1024 + ECCH], ACT.Sigmoid, bias=brgb[:])
            nc.sync.dma_start(rgb_d[:, g0 + c * ECCH:g0 + (c + 1) * ECCH], rgt[:])

    for p in reversed(ctxs):
        p.__exit__(None, None, None)


def _core_groups():
    cg = []
    for c in range(N_CORES):
        if c < 4:
            gs = [3 * c, 3 * c + 1, 3 * c + 2]
        else:
            g0 = 12 + 2 * (c - 4)
            gs = [g0, g0 + 1, g0]  # 3rd slot = dummy repeat
        cg.append(gs)
    return cg


def _prep(inputs):
    x = np.asarray(inputs["x"], dtype=np.float32)
    batch_ids = np.asarray(inputs["batch_ids"])
    perm = np.argsort(batch_ids, kind="stable")
    xs = np.ascontiguousarray(x[perm])

    xyz = xs[:, :DXYZ]
    sq = (xyz * xyz).sum(1, dtype=np.float32)

    w = {k: np.asarray(inputs[k], dtype=np.float32) for k in inputs if k not in ("x", "batch_ids")}
    e1 = w["e1_w1"]
    aw1 = np.ascontiguousarray(e1[:W + DDIR] - e1[W + DDIR:])
    cw1 = np.ascontiguousarray(e1[W + DDIR:])
    e2 = w["e2_w1"]
    a2w = np.ascontiguousarray((e2[:H] - e2[H:]) / 3.0)
    c2w = np.ascontiguousarray(e2[H:] / 3.0)

    shared = {
        "w0": w["w0"], "b0": w["b0"].reshape(W, 1),
        "wmid": w["w_mid"], "bmid": w["b_mid"].reshape(6, W, 1),
        "wskip": w["w_skip"], "bskip": w["b_skip"].reshape(W, 1),
        "wfin": w["w_final"], "bfin": w["b_final"].reshape(W, 1),
        "wsig": w["w_sigma"], "bsig": w["b_sigma"].reshape(1, 1),
        "aw1": aw1, "cw1": cw1, "ab1": w["e1_b1"].reshape(H, 1),
        "e1w2": w["e1_w2"], "e1b2": w["e1_b2"].reshape(H, 1),
        "a2w": a2w, "c2w": c2w, "ab2": w["e2_b1"].reshape(H, 1),
        "e2w2": w["e2_w2"], "e2b2": w["e2_b2"].reshape(H, 1),
        "wrgb": np.ascontiguousarray(w["w_rgb"] / 3.0), "brgb": w["b_rgb"].reshape(3, 1),
    }
    shared = {k: np.ascontiguousarray(v, dtype=np.float32) for k, v in shared.items()}

    in_maps = []
    for gs in _core_groups():
        rows = np.concatenate([np.arange(g * G, (g + 1) * G) for g in gs])
        xc = xs[rows]
        xt = np.empty((91, NODES), np.float32)
        xt[0:DXYZ] = xc[:, :DXYZ].T
        xt[DXYZ] = 1.0
        xt[DXYZ + 1:] = xc[:, DXYZ:].T
        rhsa = np.empty((64, NODES), np.float32)
        rhsa[0:DXYZ] = 2.0 * xc[:, :DXYZ].T
        rhsa[DXYZ] = -sq[rows]
        m = dict(shared)
        m["xt"] = np.ascontiguousarray(xt)
        m["rhsa"] = np.ascontiguousarray(rhsa)
        in_maps.append(m)
    return in_maps, perm


def _assemble(results, perm):
    out_sorted = np.empty((B, 4), np.float32)
    for c, gs in enumerate(_core_groups()):
        r = results[c]
        for slot, g in enumerate(gs):
            if c >= 4 and slot == 2:
                continue  # dummy
            sl = slice(slot * G, (slot + 1) * G)
            out_sorted[g * G:(g + 1) * G, 0:3] = r["rgb"][:, sl].T
            out_sorted[g * G:(g + 1) * G, 3] = r["sig"][0, sl]
    out = np.empty((B, 4), np.float32)
    out[perm] = out_sorted
    return out


def get_nc():
    if "nc" not in _STATE:
        _STATE["nc"] = _build_nc()
    return _STATE["nc"]


def kernel(**inputs) -> np.ndarray:
    nc = get_nc()
    in_maps, perm = _prep(inputs)
    res = bass_utils.run_bass_kernel_spmd(nc, in_maps, core_ids=list(range(N_CORES)))
    return _assemble(res.results, perm)


# revision 8
# speedup vs baseline: 5.7109x; 5.7109x over previous
"""Trainium2 Bass kernel for nn_NeRFGraph (gnn_message_passing).

Strategy (sharding_hint): nodes are sharded across 8 cores aligned to whole
knn batch groups. 20 groups of 2048 nodes -> cores 0-3 take 3 groups,
cores 4-7 take 2 real groups + 1 dummy (SPMD needs uniform shapes; dummy
output is dropped on the host). MLP weights are replicated (data parallel).

Per-core pipeline, per group g (all layouts are [features(partitions), nodes(free)]):
  1. kNN: scores_ij = 2*x_i.x_j - |x_j|^2 via fp32 matmuls (K=64: 63 coords +
     ones row against [2*coords; -sq]). Self always wins top-1 (score_self =
     |x_i|^2 >= score_ij), so top-3 neighbors = entries 1..3 of the DVE
     max8/max_index scan. Exact fp32 selection matches the jax reference
     (0 flips measured on the real data distribution).
  2. MLP (8 layers + skip at 4) in float32r (TF32-like, 1 cyc/row on PE).
  3. EdgeConv x2, factorized: msg_ij = relu(A_i + C_j) with A = F@(W1a-W1b)+b1,
     C = F@W1b per node; gather C_j with gpsimd ap_gather; then W2 matmul,
     mean over K=3 folded into the next layer's weights (host prescale by 1/3).
  4. rgb = sigmoid(S2 @ w_rgb/3 + b_rgb), sigma from the MLP trunk.
"""

import os
import numpy as np

import concourse.bass as bass
import concourse.tile as tile
from concourse import bacc, mybir, library_config
import concourse.bass_utils as bass_utils

F32 = mybir.dt.float32
F32R = mybir.dt.float32r
U16 = mybir.dt.uint16
I16 = mybir.dt.int16

# problem constants (hardcoded per contract)
B = 40960
NG = 20
DXYZ = 63
DDIR = 27
W = 256
H = 128  # W // 2
KNN = 3

N_CORES = 8
GPC = 3                      # groups per core (SPMD-uniform)
G = B // NG                  # 2048 nodes per group
NODES = GPC * G              # 6144 nodes per core
NT = G // 512                # node tiles of 512 per group
MT = G // 128                # row tiles of 128 per group (knn)
ECCH = 256                   # nodes per edge-conv chunk
ECM = ECCH * KNN             # messages per chunk (768)

_STATE: dict = {}


def _build_nc(reps=1):
    nc = bacc.Bacc(
        "TRN2",
        target_bir_lowering=False,
        debug=False,
        enable_asserts=False,
        num_devices=N_CORES,
    )
    d = {}

    def inp(name, shape):
        d[name] = nc.dram_tensor(name, list(shape), F32, kind="ExternalInput").ap()

    inp("xt", (91, NODES))        # rows 0-62 xyz, 63 ones, 64-90 dir
    inp("rhsa", (64, NODES))      # rows 0-62 2*xyz, 63 -sq
    inp("w0", (DXYZ, W)); inp("b0", (W, 1))
    inp("wmid", (6, W, W))        # [layer, in, out]
    inp("bmid", (6, W, 1))
    inp("wskip", (DXYZ + W, W)); inp("bskip", (W, 1))
    inp("wfin", (W, W)); inp("bfin", (W, 1))
    inp("wsig", (W, 1)); inp("bsig", (1, 1))
    inp("aw1", (W + DDIR, H)); inp("cw1", (W + DDIR, H)); inp("ab1", (H, 1))
    inp("e1w2", (H, H)); inp("e1b2", (H, 1))
    inp("a2w", (H, H)); inp("c2w", (H, H)); inp("ab2", (H, 1))
    inp("e2w2", (H, H)); inp("e2b2", (H, 1))
    inp("wrgb", (H, 3)); inp("brgb", (3, 1))

    rgb_d = nc.dram_tensor("rgb", [3, NODES], F32, kind="ExternalOutput").ap()
    sig_d = nc.dram_tensor("sig", [1, NODES], F32, kind="ExternalOutput").ap()

    with tile.TileContext(nc) as tc:
        _body(tc, d, rgb_d, sig_d, reps=reps)
    nc.compile()
    return nc


def _body(tc, d, rgb_d, sig_d, reps=1):
    nc = tc.nc
    ctxs = []

    def pool(name, bufs, space="SBUF"):
        p = tc.tile_pool(name=name, bufs=bufs, space=space)
        ctxs.append(p)
        return p.__enter__()

    wstage = pool("wstage", bufs=2)       # f32 staging for weight rounding
    wp = pool("wp", bufs=1)               # persistent rounded weights / biases
    xp = pool("xp", bufs=2)               # per-group inputs
    xr = pool("xr", bufs=1)               # rounded per-group inputs
    ap_ = pool("ap", bufs=1)              # per-group A1/C1/S1/A2/C2 tables
    ec = pool("ec", bufs=2)               # edge-conv chunk tiles
    hp = pool("hp", bufs=3)               # MLP hidden tiles [128,512]
    sp = pool("sp", bufs=4)               # small tiles (vmax/imax/bias views)
    op = pool("op", bufs=2)               # output staging
    idxp = pool("idxp", bufs=2)
    psp = pool("psp", bufs=2, space="PSUM")
    drp = pool("drp", bufs=2, space="DRAM")

    nc.gpsimd.load_library(library_config.ap_gather)

    # ---- load + round weights to f32r (one-time) ----
    # weights are stored as lists of K-chunk tiles (<=128 partitions each)
    def load_chunks(src_ap, rows, cols, tag):
        """src_ap: DRAM AP [R, cols]; returns list of rounded [r,cols] tiles."""
        out = []
        r0 = 0
        for i, r in enumerate(rows):
            st = wstage.tile([r, cols], F32, tag="wstage")
            nc.sync.dma_start(st[:], src_ap[r0:r0 + r, :])
            wt = wp.tile([r, cols], F32R, tag=f"{tag}_{i}")
            nc.vector.tensor_copy(wt[:], st[:])
            out.append(wt)
            r0 += r
        return out

    def load_b(name, src_ap, rows):
        out = []
        r0 = 0
        for i, r in enumerate(rows):
            bt = wp.tile([r, 1], F32, tag=f"{name}_{i}")
            nc.sync.dma_start(bt[:], src_ap[r0:r0 + r, :])
            out.append(bt)
            r0 += r
        return out

    w0 = load_chunks(d["w0"][:], [DXYZ], W, "w0")[0]
    wmid = [load_chunks(d["wmid"][l], [128, 128], W, f"wmid{l}") for l in range(6)]
    wskip = load_chunks(d["wskip"][:], [DXYZ, 128, 128], W, "wskip")
    wfin = load_chunks(d["wfin"][:], [128, 128], W, "wfin")
    wsig = load_chunks(d["wsig"][:], [128, 128], 1, "wsig")
    aw1 = load_chunks(d["aw1"][:], [128, 128, DDIR], H, "aw1")
    cw1 = load_chunks(d["cw1"][:], [128, 128, DDIR], H, "cw1")
    e1w2 = load_chunks(d["e1w2"][:], [H], H, "e1w2")[0]
    a2w = load_chunks(d["a2w"][:], [H], H, "a2w")[0]
    c2w = load_chunks(d["c2w"][:], [H], H, "c2w")[0]
    e2w2 = load_chunks(d["e2w2"][:], [H], H, "e2w2")[0]
    wrgb = load_chunks(d["wrgb"][:], [H], 3, "wrgb")[0]

    b0 = load_b("b0", d["b0"][:], [128, 128])
    bmid = [load_b(f"bmid{l}", d["bmid"][l], [128, 128]) for l in range(6)]
    bskip = load_b("bskip", d["bskip"][:], [128, 128])
    bfin = load_b("bfin", d["bfin"][:], [128, 128])
    bsig = load_b("bsig", d["bsig"][:], [1])[0]
    ab1 = load_b("ab1", d["ab1"][:], [H])[0]
    e1b2 = load_b("e1b2", d["e1b2"][:], [H])[0]
    ab2 = load_b("ab2", d["ab2"][:], [H])[0]
    e2b2 = load_b("e2b2", d["e2b2"][:], [H])[0]
    brgb = load_b("brgb", d["brgb"][:], [3])[0]

    ACT = mybir.ActivationFunctionType

    for g in range(GPC * reps):
        g = g % GPC
        g0 = g * G
        # ---- load per-group inputs ----
        xt = xp.tile([91, G], F32, tag="xt")
        nc.sync.dma_start(xt[:], d["xt"][:, g0:g0 + G])
        rhsa = xp.tile([64, G], F32, tag="rhsa")
        nc.sync.dma_start(rhsa[:], d["rhsa"][:, g0:g0 + G])
        xtr = xr.tile([64, G], F32R, tag="xtr")
        nc.vector.tensor_copy(xtr[:], xt[0:64, :])
        dirf = xr.tile([DDIR, G], F32, tag="dirf")
        nc.sync.dma_start(dirf[:], d["xt"][64:91, g0:g0 + G])
        dirr = xr.tile([DDIR, G], F32R, tag="dirr")
        nc.vector.tensor_copy(dirr[:], dirf[:])

        # ---- kNN ----
        nbr = drp.tile([G, KNN], U16, tag="nbr")
        for mt in range(MT):
            ps = psp.tile([128, 2048], F32, tag="ps")
            for nt in range(4):
                nc.tensor.matmul(
                    ps[:, nt * 512:(nt + 1) * 512],
                    xt[0:64, mt * 128:(mt + 1) * 128],
                    rhsa[:, nt * 512:(nt + 1) * 512],
                    start=True, stop=True,
                )
            vmax = sp.tile([128, 8], F32, tag="vmax")
            nc.vector.max(vmax[:], ps[:])
            imax = sp.tile([128, 8], U16, tag="imax")
            nc.vector.max_index(imax[:], vmax[:], ps[:])
            nc.sync.dma_start(nbr[mt * 128:(mt + 1) * 128, :], imax[:, 1:4])

        # wrapped gather indices: element m lives at [m % 16, m // 16],
        # replicated into each 16-partition block (one per Q7 core)
        idxw = idxp.tile([128, G * KNN // 16], I16, tag="idxw")
        nbr_w = nbr[:].rearrange("n k -> (n k)").rearrange("(f p) -> p f", p=16)
        for r in range(8):
            nc.sync.dma_start(idxw[16 * r:16 * r + 16, :], nbr_w.bitcast(I16))

        # ---- MLP + A1/C1/sigma ----
        a1 = ap_.tile([H, G], F32, tag="a1")
        c1 = ap_.tile([H, G], F32, tag="c1")
        for nt in range(NT):
            n0 = nt * 512
            sl = slice(n0, n0 + 512)

            def evac(ps, src_sl, dst, bias, func):
                nc.scalar.activation(dst, ps[:, src_sl], func, bias=bias)

            # L0: [63]->256
            ps = psp.tile([128, 2048], F32, tag="ps")
            nc.tensor.matmul(ps[:, 0:512], w0[:, 0:128], xtr[0:DXYZ, sl], start=True, stop=True)
            nc.tensor.matmul(ps[:, 512:1024], w0[:, 128:256], xtr[0:DXYZ, sl], start=True, stop=True)
            h = [hp.tile([128, 512], F32R, tag=f"h{mh}", name=f"h{mh}") for mh in range(2)]
            evac(ps, slice(0, 512), h[0][:], b0[0][:], ACT.Relu)
            evac(ps, slice(512, 1024), h[1][:], b0[1][:], ACT.Relu)

            # layers 1..7
            m = 0
            for layer in range(1, 8):
                ps = psp.tile([128, 2048], F32, tag="ps")
                if layer == 4:
                    bk = bskip
                    for mh in range(2):
                        osl = slice(mh * 512, mh * 512 + 512)
                        msl = slice(mh * 128, mh * 128 + 128)
                        nc.tensor.matmul(ps[:, osl], wskip[0][:, msl],
                                         xtr[0:DXYZ, sl], start=True, stop=False)
                        nc.tensor.matmul(ps[:, osl], wskip[1][:, msl],
                                         h[0][:], start=False, stop=False)
                        nc.tensor.matmul(ps[:, osl], wskip[2][:, msl],
                                         h[1][:], start=False, stop=True)
                else:
                    wk, bk = wmid[m], bmid[m]
                    m += 1
                    for mh in range(2):
                        osl = slice(mh * 512, mh * 512 + 512)
                        msl = slice(mh * 128, mh * 128 + 128)
                        nc.tensor.matmul(ps[:, osl], wk[0][:, msl],
                                         h[0][:], start=True, stop=False)
                        nc.tensor.matmul(ps[:, osl], wk[1][:, msl],
                                         h[1][:], start=False, stop=True)
                hn = [hp.tile([128, 512], F32R, tag=f"h{mh}", name=f"hn{mh}") for mh in range(2)]
                evac(ps, slice(0, 512), hn[0][:], bk[0][:], ACT.Relu)
                evac(ps, slice(512, 1024), hn[1][:], bk[1][:], ACT.Relu)
                h = hn

            # final (no relu) + sigma
            ps = psp.tile([128, 2048], F32, tag="ps")
            for mh in range(2):
                osl = slice(mh * 512, mh * 512 + 512)
                msl = slice(mh * 128, mh * 128 + 128)
                nc.tensor.matmul(ps[:, osl], wfin[0][:, msl],
                                 h[0][:], start=True, stop=False)
                nc.tensor.matmul(ps[:, osl], wfin[1][:, msl],
                                 h[1][:], start=False, stop=True)
            nc.tensor.matmul(ps[0:1, 1024:1536], wsig[0][:], h[0][:], start=True, stop=False)
            nc.tensor.matmul(ps[0:1, 1024:1536], wsig[1][:], h[1][:], start=False, stop=True)
            feat = [hp.tile([128, 512], F32R, tag=f"feat{mh}", name=f"feat{mh}") for mh in range(2)]
            evac(ps, slice(0, 512), feat[0][:], bfin[0][:], ACT.Identity)
            evac(ps, slice(512, 1024), feat[1][:], bfin[1][:], ACT.Identity)
            sgt = op.tile([1, 512], F32, tag="sgt")
            nc.scalar.activation(sgt[:], ps[0:1, 1024:1536], ACT.Identity, bias=bsig[:])
            nc.sync.dma_start(sig_d[:, g0 + n0:g0 + n0 + 512], sgt[:])

            # A1 / C1 over feat(256) + dir(27)
            ps = psp.tile([128, 2048], F32, tag="ps")
            for dst_sl, wt in ((slice(0, 512), aw1), (slice(512, 1024), cw1)):
                nc.tensor.matmul(ps[:, dst_sl], wt[0][:], feat[0][:], start=True, stop=False)
                nc.tensor.matmul(ps[:, dst_sl], wt[1][:], feat[1][:], start=False, stop=False)
                nc.tensor.matmul(ps[:, dst_sl], wt[2][:], dirr[:, sl], start=False, stop=True)
            nc.scalar.activation(a1[:, sl], ps[:, 0:512], ACT.Identity, bias=ab1[:])
            nc.scalar.activation(c1[:, sl], ps[:, 512:1024], ACT.Copy)

        # ---- EdgeConv 1 ----
        s1 = ap_.tile([H, G], F32R, tag="s1")
        for c in range(G // ECCH):
            isl = slice(c * (ECM // 16), (c + 1) * (ECM // 16))
            nsl = slice(c * ECCH, (c + 1) * ECCH)
            g1 = ec.tile([128, ECM], F32, tag="g1")
            nc.gpsimd.ap_gather(g1[:], c1[:], idxw[:, isl], channels=128,
                                num_elems=G, d=1, num_idxs=ECM)
            msg = ec.tile([128, ECM], F32, tag="msg")
            g3 = g1[:].rearrange("p (n k) -> p n k", k=KNN)
            m3 = msg[:].rearrange("p (n k) -> p n k", k=KNN)
            for k in range(KNN):
                nc.vector.tensor_add(m3[:, :, k], g3[:, :, k], a1[:, nsl])
            msgr = ec.tile([128, ECM], F32R, tag="msgr")
            nc.scalar.activation(msgr[:], msg[:], ACT.Relu)
            ps = psp.tile([128, 2048], F32, tag="ps")
            nc.tensor.matmul(ps[:, 0:512], e1w2[:], msgr[:, 0:512], start=True, stop=True)
            nc.tensor.matmul(ps[:, 512:512 + ECM - 512], e1w2[:], msgr[:, 512:ECM], start=True, stop=True)
            h2 = ec.tile([128, ECM], F32, tag="h2")
            nc.scalar.activation(h2[:], ps[:, 0:ECM], ACT.Relu, bias=e1b2[:])
            h23 = h2[:].rearrange("p (n k) -> p n k", k=KNN)
            tmp = ec.tile([128, ECCH], F32, tag="trio")
            nc.vector.tensor_add(tmp[:], h23[:, :, 0], h23[:, :, 1])
            nc.vector.tensor_add(s1[:, nsl], tmp[:], h23[:, :, 2])

        # ---- EdgeConv 2 ----
        a2 = ap_.tile([H, G], F32, tag="a2")
        c2 = ap_.tile([H, G], F32, tag="c2")
        for c in range(NT):
            nsl = slice(c * 512, (c + 1) * 512)
            ps = psp.tile([128, 2048], F32, tag="ps")
            nc.tensor.matmul(ps[:, 0:512], a2w[:], s1[:, nsl], start=True, stop=True)
            nc.tensor.matmul(ps[:, 512:1024], c2w[:], s1[:, nsl], start=True, stop=True)
            nc.scalar.activation(a2[:, nsl], ps[:, 0:512], ACT.Identity, bias=ab2[:])
            nc.scalar.activation(c2[:, nsl], ps[:, 512:1024], ACT.Copy)

        for c in range(G // ECCH):
            isl = slice(c * (ECM // 16), (c + 1) * (ECM // 16))
            nsl = slice(c * ECCH, (c + 1) * ECCH)
            g2 = ec.tile([128, ECM], F32, tag="g1")
            nc.gpsimd.ap_gather(g2[:], c2[:], idxw[:, isl], channels=128,
                                num_elems=G, d=1, num_idxs=ECM)
            msg = ec.tile([128, ECM], F32, tag="msg")
            g3 = g2[:].rearrange("p (n k) -> p n k", k=KNN)
            m3 = msg[:].rearrange("p (n k) -> p n k", k=KNN)
            for k in range(KNN):
                nc.vector.tensor_add(m3[:, :, k], g3[:, :, k], a2[:, nsl])
            msgr = ec.tile([128, ECM], F32R, tag="msgr")
            nc.scalar.activation(msgr[:], msg[:], ACT.Relu)
            ps = psp.tile([128, 2048], F32, tag="ps")
            nc.tensor.matmul(ps[:, 0:512], e2w2[:], msgr[:, 0:512], start=True, stop=True)
            nc.tensor.matmul(ps[:, 512:512 + ECM - 512], e2w2[:], msgr[:, 512:ECM], start=True, stop=True)
            h2 = ec.tile([128, ECM], F32, tag="h2")
            nc.scalar.activation(h2[:], ps[:, 0:ECM], ACT.Relu, bias=e2b2[:])
            h23 = h2[:].rearrange("p (n k) -> p n k", k=KNN)
            tmp = ec.tile([128, ECCH], F32, tag="trio")
            nc.vector.tensor_add(tmp[:], h23[:, :, 0], h23[:, :, 1])
            s2 = ec.tile([128, ECCH], F32R, tag="s2")
            nc.vector.tensor_add(s2[:], tmp[:], h23[:, :, 2])
            # rgb (same psum tile, different banks than the W2 region)
            nc.tensor.matmul(ps[0:3, 1024:1024 + ECCH], wrgb[:], s2[:], start=True, stop=True)
            rgt = op.tile([3, ECCH], F32, tag="rgt")
            nc.scalar.activation(rgt[:], ps[0:3, 1024:1024 + ECCH], ACT.Sigmoid, bias=brgb[:])
            nc.sync.dma_start(rgb_d[:, g0 + c * ECCH:g0 + (c + 1) * ECCH], rgt[:])

    for p in reversed(ctxs):
        p.__exit__(None, None, None)


def _core_groups():
    cg = []
    for c in range(N_CORES):
        if c < 4:
            gs = [3 * c, 3 * c + 1, 3 * c + 2]
        else:
            g0 = 12 + 2 * (c - 4)
            gs = [g0, g0 + 1, g0]  # 3rd slot = dummy repeat
        cg.append(gs)
    return cg


def _prep(inputs):
    x = np.asarray(inputs["x"], dtype=np.float32)
    batch_ids = np.asarray(inputs["batch_ids"])
    perm = np.argsort(batch_ids, kind="stable")
    xs = np.ascontiguousarray(x[perm])

    xyz = xs[:, :DXYZ]
    sq = (xyz * xyz).sum(1, dtype=np.float32)

    w = {k: np.asarray(inputs[k], dtype=np.float32) for k in inputs if k not in ("x", "batch_ids")}
    e1 = w["e1_w1"]
    aw1 = np.ascontiguousarray(e1[:W + DDIR] - e1[W + DDIR:])
    cw1 = np.ascontiguousarray(e1[W + DDIR:])
    e2 = w["e2_w1"]
    a2w = np.ascontiguousarray((e2[:H] - e2[H:]) / 3.0)
    c2w = np.ascontiguousarray(e2[H:] / 3.0)

    shared = {
        "w0": w["w0"], "b0": w["b0"].reshape(W, 1),
        "wmid": w["w_mid"], "bmid": w["b_mid"].reshape(6, W, 1),
        "wskip": w["w_skip"], "bskip": w["b_skip"].reshape(W, 1),
        "wfin": w["w_final"], "bfin": w["b_final"].reshape(W, 1),
        "wsig": w["w_sigma"], "bsig": w["b_sigma"].reshape(1, 1),
        "aw1": aw1, "cw1": cw1, "ab1": w["e1_b1"].reshape(H, 1),
        "e1w2": w["e1_w2"], "e1b2": w["e1_b2"].reshape(H, 1),
        "a2w": a2w, "c2w": c2w, "ab2": w["e2_b1"].reshape(H, 1),
        "e2w2": w["e2_w2"], "e2b2": w["e2_b2"].reshape(H, 1),
        "wrgb": np.ascontiguousarray(w["w_rgb"] / 3.0), "brgb": w["b_rgb"].reshape(3, 1),
    }
    shared = {k: np.ascontiguousarray(v, dtype=np.float32) for k, v in shared.items()}

    in_maps = []
    for gs in _core_groups():
        rows = np.concatenate([np.arange(g * G, (g + 1) * G) for g in gs])
        xc = xs[rows]
        xt = np.empty((91, NODES), np.float32)
        xt[0:DXYZ] = xc[:, :DXYZ].T
        xt[DXYZ] = 1.0
        xt[DXYZ + 1:] = xc[:, DXYZ:].T
        rhsa = np.empty((64, NODES), np.float32)
        rhsa[0:DXYZ] = 2.0 * xc[:, :DXYZ].T
        rhsa[DXYZ] = -sq[rows]
        m = dict(shared)
        m["xt"] = np.ascontiguousarray(xt)
        m["rhsa"] = np.ascontiguousarray(rhsa)
        in_maps.append(m)
    return in_maps, perm


def _assemble(results, perm):
    out_sorted = np.empty((B, 4), np.float32)
    for c, gs in enumerate(_core_groups()):
        r = results[c]
        for slot, g in enumerate(gs):
            if c >= 4 and slot == 2:
                continue  # dummy
            sl = slice(slot * G, (slot + 1) * G)
            out_sorted[g * G:(g + 1) * G, 0:3] = r["rgb"][:, sl].T
            out_sorted[g * G:(g + 1) * G, 3] = r["sig"][0, sl]
    out = np.empty((B, 4), np.float32)
    out[perm] = out_sorted
    return out


def get_nc(reps=1):
    key = f"nc{reps}"
    if key not in _STATE:
        _STATE[key] = _build_nc(reps)
    return _STATE[key]


def kernel(**inputs) -> np.ndarray:
    nc = get_nc()
    in_maps, perm = _prep(inputs)
    res = bass_utils.run_bass_kernel_spmd(nc, in_maps, core_ids=list(range(N_CORES)))
    return _assemble(res.results, perm)
